# revision 1
# baseline (speedup 1.0000x reference)
"""RBF kernel attention (nn_KernelAttention) on 8 Trainium2 NeuronCores.

reference math (per batch b):
    dist2[i,j] = ||x_i||^2 + ||x_j||^2 - 2 x_i.x_j
    attn = softmax(-gamma * max(dist2, 0), axis=j)
    out  = attn @ x

Key structural facts used here:
  * For RBF attention the diagonal logit is always ~0 and all logits are
    <= 0 (dist2 >= 0), so no separate row-max pass is needed for a safe
    exp() -- we exponentiate -gamma*dist2 directly and normalize by the
    row sum, exactly like the reference (which subtracts a row max of 0).
  * softmax is invariant to a per-row (per-query) additive constant, so
    the -gamma*||x_q||^2 term only needs enough precision to prevent
    overflow; bf16 is plenty (any rounding cancels between P and sum(P)).
  * We compute the score matrix transposed, L^T[k, q], so the exp output
    P^T is directly the stationary (lhsT) operand of the P @ V matmul --
    no on-chip transpose of the attention matrix is ever needed.

Sharding: core c handles batch c//2, query half c%2 (2048 queries),
against the batch's full 4096 keys. No collectives; host concatenates.

SPMD trick: every core receives x_self (its own query rows, also the
first half of its key/value set) and x_other (the remaining rows).
Attention is key-order invariant, so "self keys first" is fine and all
cores run the identical program.
"""

import sys

if "/opt/trn_rl_repo" not in sys.path:
    sys.path.insert(0, "/opt/trn_rl_repo")

from contextlib import ExitStack

import numpy as np

import concourse.bass as bass
import concourse.mybir as mybir
import concourse.tile as tile
from concourse import bacc
from concourse.bass_utils import run_bass_kernel_spmd

F32 = mybir.dt.float32
BF16 = mybir.dt.bfloat16
AF = mybir.ActivationFunctionType

B, S, E = 4, 4096, 1024
NCORES = 8
P = 128                 # partitions
SQ = S // 2             # queries per core
NKB = S // P            # 32 key blocks
NKB_SELF = SQ // P      # 16 key blocks coming from x_self
NEC = E // P            # 8 contraction chunks for Q@K^T
QB = 512                # query free-dim tile for QK / exp
NQB = SQ // QB          # 4
NQS = QB // P           # 4 query subtiles per query block
EH = 512                # PV free-dim half (PSUM bank limit)


def _build_body(ctx: ExitStack, tc: tile.TileContext, gamma: float,
                xs_d, xo_d, out_d, sqq_d):
    nc = tc.nc

    const = ctx.enter_context(tc.tile_pool(name="const", bufs=1))
    stage = ctx.enter_context(tc.tile_pool(name="stage", bufs=4))
    tpool = ctx.enter_context(tc.tile_pool(name="tpool", bufs=3))
    opool = ctx.enter_context(tc.tile_pool(name="opool", bufs=2))
    small = ctx.enter_context(tc.tile_pool(name="small", bufs=2))
    ptp = ctx.enter_context(tc.tile_pool(name="ptp", bufs=1))

    # ---- persistent SBUF tiles ----
    xT = [const.tile([P, S], BF16, name=f"xT{e}", tag=f"xT{e}")
          for e in range(NEC)]                       # [E-chunk][e_part, k]
    V = [const.tile([P, E], BF16, name=f"V{kb}", tag=f"V{kb}")
         for kb in range(NKB)]                       # [k-block][k_part, e]
    sq_all = const.tile([P, NKB], F32, name="sq_all", tag="sq_all")
    biasK = const.tile([P, NKB], F32, name="biasK", tag="biasK")
    sqq_sc = const.tile([P, NKB_SELF], BF16, name="sqq_sc", tag="sqq_sc")
    bcastQ = const.tile([P, SQ], BF16, name="bcastQ", tag="bcastQ")
    ones = const.tile([P, 1], BF16, name="ones", tag="ones")
    nc.vector.memset(ones, 1.0)

    # ---- prologue: load x, compute ||x||^2, cast to bf16, build x^T ----
    with tc.tile_pool(name="sq_ps", bufs=2, space="PSUM") as sq_ps:
        for kb in range(NKB):
            src = xs_d if kb < NKB_SELF else xo_d
            r0 = (kb % NKB_SELF) * P
            xst = stage.tile([P, E], F32, name="xst", tag="xst")
            nc.sync.dma_start(out=xst, in_=src[r0:r0 + P, :])
            # sum of squares per row via ACT accumulate (squares discarded)
            sqt = sq_ps.tile([P, E], F32, name="sqt", tag="sqt")
            nc.scalar.activation(sqt, xst, AF.Square,
                                 accum_out=sq_all[:, kb:kb + 1])
            nc.scalar.copy(V[kb], xst)               # f32 -> bf16 cast
            for e in range(NEC):
                nc.sync.dma_start_transpose(
                    out=xT[e][:, kb * P:(kb + 1) * P],
                    in_=V[kb][:, e * P:(e + 1) * P])

    nc.vector.tensor_scalar_mul(biasK, sq_all, -gamma)
    nc.vector.tensor_scalar_mul(sqq_sc, sq_all[:, :NKB_SELF], -0.5)
    # roundtrip through DRAM to broadcast -0.5*||x_q||^2 along partitions
    nc.sync.dma_start(out=sqq_d[:].rearrange("(c p) -> p c", p=P), in_=sqq_sc)
    s_ap = sqq_d[:]
    bq_src = bass.AP(tensor=s_ap.tensor, offset=s_ap.offset,
                     ap=[[0, P]] + list(s_ap.ap))
    nc.sync.dma_start(out=bcastQ, in_=bq_src)

    # ---- main loop: PSUM pools (8 banks total: 2 + 4 + 2) ----
    qk_ps = ctx.enter_context(tc.tile_pool(name="qk_ps", bufs=2, space="PSUM"))
    out_ps = ctx.enter_context(tc.tile_pool(name="out_ps", bufs=2, space="PSUM"))
    s_ps = ctx.enter_context(tc.tile_pool(name="s_ps", bufs=2, space="PSUM"))

    for qb in range(NQB):
        q0 = qb * QB
        # Phase A: P^T[k, q0:q0+QB] for all 32 key blocks
        pts = []
        for kb in range(NKB):
            qkp = qk_ps.tile([P, QB], F32, name="qkp", tag="qkp")
            for e in range(NEC):
                nc.tensor.matmul(qkp,
                                 lhsT=xT[e][:, kb * P:(kb + 1) * P],
                                 rhs=xT[e][:, q0:q0 + QB],
                                 start=(e == 0), stop=(e == NEC - 1))
            tt = tpool.tile([P, QB], F32, name="tt", tag="tt")
            nc.vector.tensor_add(tt, qkp, bcastQ[:, q0:q0 + QB])
            pt = ptp.tile([P, QB], BF16, name=f"pt{kb}", tag=f"pt{kb}")
            nc.scalar.activation(pt, tt, AF.Exp,
                                 bias=biasK[:, kb:kb + 1], scale=2.0 * gamma)
            pts.append(pt)
        # Phase B: out[q, :] = (P^T)^T @ V, row-sum via ones column
        for qs in range(NQS):
            po = out_ps.tile([P, E], F32, name="po", tag="po")
            sp = s_ps.tile([P, 1], F32, name="sp", tag="sp")
            for kb in range(NKB):
                lw = pts[kb][:, qs * P:(qs + 1) * P]
                nc.tensor.matmul(po[:, 0:EH], lhsT=lw, rhs=V[kb][:, 0:EH],
                                 start=(kb == 0), stop=(kb == NKB - 1))
                nc.tensor.matmul(po[:, EH:E], lhsT=lw, rhs=V[kb][:, EH:E],
                                 start=(kb == 0), stop=(kb == NKB - 1))
                nc.tensor.matmul(sp, lhsT=lw, rhs=ones,
                                 start=(kb == 0), stop=(kb == NKB - 1))
            rc = small.tile([P, 1], F32, name="rc", tag="rc")
            nc.vector.reciprocal(rc, sp)
            ot = opool.tile([P, E], F32, name="ot", tag="ot")
            nc.vector.tensor_scalar_mul(ot, po, rc)
            nc.sync.dma_start(out=out_d[q0 + qs * P:q0 + (qs + 1) * P, :],
                              in_=ot)


def build_module(gamma: float):
    nc = bacc.Bacc("TRN2", target_bir_lowering=False, debug=False)
    xs_d = nc.dram_tensor("x_self", [SQ, E], F32, kind="ExternalInput")
    xo_d = nc.dram_tensor("x_other", [SQ, E], F32, kind="ExternalInput")
    out_d = nc.dram_tensor("out", [SQ, E], F32, kind="ExternalOutput")
    sqq_d = nc.dram_tensor("sqq_scratch", [SQ], BF16)
    with tile.TileContext(nc) as tc, ExitStack() as ctx:
        _build_body(ctx, tc, gamma, xs_d, xo_d, out_d, sqq_d)
    nc.compile()
    return nc


_CACHE: dict[float, object] = {}


def _get_module(gamma: float):
    if gamma not in _CACHE:
        _CACHE[gamma] = build_module(gamma)
    return _CACHE[gamma]


def kernel(x, gamma):
    x = np.ascontiguousarray(np.asarray(x, dtype=np.float32))
    g = float(np.asarray(gamma))
    nc = _get_module(g)
    in_maps = []
    for c in range(NCORES):
        b, h = divmod(c, 2)
        xs = np.ascontiguousarray(x[b, h * SQ:(h + 1) * SQ])
        xo = np.ascontiguousarray(x[b, (1 - h) * SQ:(2 - h) * SQ])
        in_maps.append({"x_self": xs, "x_other": xo})
    res = run_bass_kernel_spmd(nc, in_maps, list(range(NCORES))).results
    out = np.empty((B, S, E), np.float32)
    for c in range(NCORES):
        b, h = divmod(c, 2)
        out[b, h * SQ:(h + 1) * SQ] = res[c]["out"]
    return out


if __name__ == "__main__":
    xs = np.random.randn(B, S, E).astype(np.float32)
    o = kernel(xs, np.float32(1.0))
    print("ran", o.shape, o.dtype)


# revision 3
# speedup vs baseline: 1.7257x; 1.7257x over previous
"""RBF kernel attention (nn_KernelAttention) on 8 Trainium2 NeuronCores.

reference math (per batch b):
    dist2[i,j] = ||x_i||^2 + ||x_j||^2 - 2 x_i.x_j
    attn = softmax(-gamma * max(dist2, 0), axis=j)
    out  = attn @ x

Key structural facts used here:
  * For RBF attention the diagonal logit is always ~0 and all logits are
    <= 0 (dist2 >= 0), so no separate row-max pass is needed for a safe
    exp() -- we exponentiate -gamma*dist2 directly and normalize by the
    row sum, exactly like the reference (which subtracts a row max of 0).
  * softmax is invariant to a per-row (per-query) additive constant, so
    the -gamma*||x_q||^2 term only needs enough precision to prevent
    overflow; bf16 is plenty (any rounding cancels between P and sum(P)).
  * We compute the score matrix transposed, L^T[k, q], so the exp output
    P^T is directly the stationary (lhsT) operand of the P @ V matmul --
    no on-chip transpose of the attention matrix is ever needed.

Sharding: core c handles batch c//2, query half c%2 (2048 queries),
against the batch's full 4096 keys. No collectives; host concatenates.

SPMD trick: every core receives x_self (its own query rows, also the
first half of its key/value set) and x_other (the remaining rows).
Attention is key-order invariant, so "self keys first" is fine and all
cores run the identical program.
"""

import sys

if "/opt/trn_rl_repo" not in sys.path:
    sys.path.insert(0, "/opt/trn_rl_repo")

from contextlib import ExitStack

import numpy as np

import concourse.bass as bass
import concourse.mybir as mybir
import concourse.tile as tile
from concourse import bacc
from concourse.bass_utils import run_bass_kernel_spmd
from concourse.masks import make_identity

F32 = mybir.dt.float32
BF16 = mybir.dt.bfloat16
AF = mybir.ActivationFunctionType

B, S, E = 4, 4096, 1024
NCORES = 8
P = 128                 # partitions
SQ = S // 2             # queries per core
NKB = S // P            # 32 key blocks
NKB_SELF = SQ // P      # 16 key blocks coming from x_self
NEC = E // P            # 8 contraction chunks for Q@K^T
QB = 512                # query free-dim tile for QK / exp
NQB = SQ // QB          # 4
NQS = QB // P           # 4 query subtiles per query block
EH = 512                # PV free-dim half (PSUM bank limit)


def _build_body(ctx: ExitStack, tc: tile.TileContext, gamma: float,
                xs_d, xo_d, out_d, sqq_d):
    nc = tc.nc

    const = ctx.enter_context(tc.tile_pool(name="const", bufs=1))
    stage = ctx.enter_context(tc.tile_pool(name="stage", bufs=4))
    tpool = ctx.enter_context(tc.tile_pool(name="tpool", bufs=3))
    opool = ctx.enter_context(tc.tile_pool(name="opool", bufs=2))
    small = ctx.enter_context(tc.tile_pool(name="small", bufs=2))
    ptp = ctx.enter_context(tc.tile_pool(name="ptp", bufs=1))

    # ---- persistent SBUF tiles ----
    xT = [const.tile([P, S], BF16, name=f"xT{e}", tag=f"xT{e}")
          for e in range(NEC)]                       # [E-chunk][e_part, k]
    V = [const.tile([P, E], BF16, name=f"V{kb}", tag=f"V{kb}")
         for kb in range(NKB)]                       # [k-block][k_part, e]
    sq_all = const.tile([P, NKB], F32, name="sq_all", tag="sq_all")
    biasK = const.tile([P, NKB], F32, name="biasK", tag="biasK")
    sqq_sc = const.tile([P, NKB_SELF], BF16, name="sqq_sc", tag="sqq_sc")
    bcastQ = const.tile([P, SQ], BF16, name="bcastQ", tag="bcastQ")
    ones = const.tile([P, 1], BF16, name="ones", tag="ones")
    nc.vector.memset(ones, 1.0)
    ident = const.tile([P, P], BF16, name="ident", tag="ident")
    make_identity(nc, ident)

    # ---- prologue: load x, compute ||x||^2, cast to bf16, build x^T ----
    # Transposes run on the (otherwise idle) PE via identity matmuls; DVE
    # drains them from PSUM into the xT tiles.
    with tc.tile_pool(name="sq_ps", bufs=2, space="PSUM") as sq_ps, \
         tc.tile_pool(name="tr_ps", bufs=4, space="PSUM") as tr_ps:
        for kb in range(NKB):
            src = xs_d if kb < NKB_SELF else xo_d
            r0 = (kb % NKB_SELF) * P
            xst = stage.tile([P, E], F32, name="xst", tag="xst")
            nc.sync.dma_start(out=xst, in_=src[r0:r0 + P, :])
            # sum of squares per row via ACT accumulate (squares discarded)
            sqt = sq_ps.tile([P, E], F32, name="sqt", tag="sqt")
            nc.scalar.activation(sqt, xst, AF.Square,
                                 accum_out=sq_all[:, kb:kb + 1])
            nc.scalar.copy(V[kb], xst)               # f32 -> bf16 cast
            for e in range(NEC):
                trp = tr_ps.tile([P, P], BF16, name="trp", tag="trp")
                nc.tensor.transpose(trp, V[kb][:, e * P:(e + 1) * P], ident)
                nc.vector.tensor_copy(xT[e][:, kb * P:(kb + 1) * P], trp)

    nc.vector.tensor_scalar_mul(biasK, sq_all, -gamma)
    nc.vector.tensor_scalar_mul(sqq_sc, sq_all[:, :NKB_SELF], -0.5)
    # roundtrip through DRAM to broadcast -0.5*||x_q||^2 along partitions
    nc.sync.dma_start(out=sqq_d[:].rearrange("(c p) -> p c", p=P), in_=sqq_sc)
    s_ap = sqq_d[:]
    bq_src = bass.AP(tensor=s_ap.tensor, offset=s_ap.offset,
                     ap=[[0, P]] + list(s_ap.ap))
    nc.sync.dma_start(out=bcastQ, in_=bq_src)

    # ---- main loop: PSUM pools (8 banks total: 2 + 4 + 2) ----
    qk_ps = ctx.enter_context(tc.tile_pool(name="qk_ps", bufs=2, space="PSUM"))
    out_ps = ctx.enter_context(tc.tile_pool(name="out_ps", bufs=2, space="PSUM"))
    s_ps = ctx.enter_context(tc.tile_pool(name="s_ps", bufs=2, space="PSUM"))

    for qb in range(NQB):
        q0 = qb * QB
        # Phase A: P^T[k, q0:q0+QB] for all 32 key blocks
        pts = []
        for kb in range(NKB):
            qkp = qk_ps.tile([P, QB], F32, name="qkp", tag="qkp")
            for e in range(NEC):
                nc.tensor.matmul(qkp,
                                 lhsT=xT[e][:, kb * P:(kb + 1) * P],
                                 rhs=xT[e][:, q0:q0 + QB],
                                 start=(e == 0), stop=(e == NEC - 1))
            tt = tpool.tile([P, QB], F32, name="tt", tag="tt")
            nc.vector.tensor_add(tt, qkp, bcastQ[:, q0:q0 + QB])
            pt = ptp.tile([P, QB], BF16, name=f"pt{kb}", tag=f"pt{kb}")
            nc.scalar.activation(pt, tt, AF.Exp,
                                 bias=biasK[:, kb:kb + 1], scale=2.0 * gamma)
            pts.append(pt)
        # Phase B: out[q, :] = (P^T)^T @ V, row-sum via ones column
        for qs in range(NQS):
            po = out_ps.tile([P, E], F32, name="po", tag="po")
            sp = s_ps.tile([P, 1], F32, name="sp", tag="sp")
            for kb in range(NKB):
                lw = pts[kb][:, qs * P:(qs + 1) * P]
                nc.tensor.matmul(po[:, 0:EH], lhsT=lw, rhs=V[kb][:, 0:EH],
                                 start=(kb == 0), stop=(kb == NKB - 1))
                nc.tensor.matmul(po[:, EH:E], lhsT=lw, rhs=V[kb][:, EH:E],
                                 start=(kb == 0), stop=(kb == NKB - 1))
                nc.tensor.matmul(sp, lhsT=lw, rhs=ones,
                                 start=(kb == 0), stop=(kb == NKB - 1))
            rc = small.tile([P, 1], F32, name="rc", tag="rc")
            nc.vector.reciprocal(rc, sp)
            ot = opool.tile([P, E], F32, name="ot", tag="ot")
            nc.vector.tensor_scalar_mul(ot, po, rc)
            nc.sync.dma_start(out=out_d[q0 + qs * P:q0 + (qs + 1) * P, :],
                              in_=ot)


def build_module(gamma: float):
    nc = bacc.Bacc("TRN2", target_bir_lowering=False, debug=False)
    xs_d = nc.dram_tensor("x_self", [SQ, E], F32, kind="ExternalInput")
    xo_d = nc.dram_tensor("x_other", [SQ, E], F32, kind="ExternalInput")
    out_d = nc.dram_tensor("out", [SQ, E], F32, kind="ExternalOutput")
    sqq_d = nc.dram_tensor("sqq_scratch", [SQ], BF16)
    with tile.TileContext(nc) as tc, ExitStack() as ctx:
        _build_body(ctx, tc, gamma, xs_d, xo_d, out_d, sqq_d)
    nc.compile()
    return nc


_CACHE: dict[float, object] = {}


def _get_module(gamma: float):
    if gamma not in _CACHE:
        _CACHE[gamma] = build_module(gamma)
    return _CACHE[gamma]


def kernel(x, gamma):
    x = np.ascontiguousarray(np.asarray(x, dtype=np.float32))
    g = float(np.asarray(gamma))
    nc = _get_module(g)
    in_maps = []
    for c in range(NCORES):
        b, h = divmod(c, 2)
        xs = np.ascontiguousarray(x[b, h * SQ:(h + 1) * SQ])
        xo = np.ascontiguousarray(x[b, (1 - h) * SQ:(2 - h) * SQ])
        in_maps.append({"x_self": xs, "x_other": xo})
    res = run_bass_kernel_spmd(nc, in_maps, list(range(NCORES))).results
    out = np.empty((B, S, E), np.float32)
    for c in range(NCORES):
        b, h = divmod(c, 2)
        out[b, h * SQ:(h + 1) * SQ] = res[c]["out"]
    return out


if __name__ == "__main__":
    xs = np.random.randn(B, S, E).astype(np.float32)
    o = kernel(xs, np.float32(1.0))
    print("ran", o.shape, o.dtype)


# revision 7
# speedup vs baseline: 2.1230x; 1.2302x over previous
"""RBF kernel attention (nn_KernelAttention) on 8 Trainium2 NeuronCores.

reference math (per batch b):
    dist2[i,j] = ||x_i||^2 + ||x_j||^2 - 2 x_i.x_j
    attn = softmax(-gamma * max(dist2, 0), axis=j)
    out  = attn @ x

Key structural facts used here:
  * For RBF attention the diagonal logit is always ~0 and all logits are
    <= 0 (dist2 >= 0), so no separate row-max pass is needed for a safe
    exp() -- we exponentiate -gamma*dist2 directly and normalize by the
    row sum, exactly like the reference (which subtracts a row max of 0).
  * softmax is invariant to a per-row (per-query) additive constant, so
    the -gamma*||x_q||^2 term only needs enough precision to prevent
    overflow; bf16 is plenty (any rounding cancels between P and sum(P)).
  * We compute the score matrix transposed, L^T[k, q], so the exp output
    P^T is directly the stationary (lhsT) operand of the P @ V matmul --
    no on-chip transpose of the attention matrix is ever needed.

Sharding: core c handles batch c//2, query half c%2 (2048 queries),
against the batch's full 4096 keys. No collectives; host concatenates.

SPMD trick: every core receives x_self (its own query rows, also the
first half of its key/value set) and x_other (the remaining rows).
Attention is key-order invariant, so "self keys first" is fine and all
cores run the identical program.
"""

import sys

if "/opt/trn_rl_repo" not in sys.path:
    sys.path.insert(0, "/opt/trn_rl_repo")

from contextlib import ExitStack

import numpy as np

import concourse.bass as bass
import concourse.mybir as mybir
import concourse.tile as tile
from concourse import bacc
from concourse.bass_utils import run_bass_kernel_spmd
from concourse.masks import make_identity

F32 = mybir.dt.float32
BF16 = mybir.dt.bfloat16
FP8 = mybir.dt.float8e4
AF = mybir.ActivationFunctionType

FP8_QK = True   # fp8 DoubleRow for the Q@K^T gram matmul (2x PE rate)

B, S, E = 4, 4096, 1024
NCORES = 8
P = 128                 # partitions
SQ = S // 2             # queries per core
NKB = S // P            # 32 key blocks
NKB_SELF = SQ // P      # 16 key blocks coming from x_self
NEC = E // P            # 8 contraction chunks for Q@K^T
QB = 512                # query free-dim tile for QK / exp
NQB = SQ // QB          # 4
NQS = QB // P           # 4 query subtiles per query block
EH = 512                # PV free-dim half (PSUM bank limit)


def _build_body(ctx: ExitStack, tc: tile.TileContext, gamma: float,
                xs_d, xo_d, out_d, sqq_d):
    nc = tc.nc

    const = ctx.enter_context(tc.tile_pool(name="const", bufs=1))
    stage = ctx.enter_context(tc.tile_pool(name="stage", bufs=4))
    tpool = ctx.enter_context(tc.tile_pool(name="tpool", bufs=3))
    opool = ctx.enter_context(tc.tile_pool(name="opool", bufs=2))
    small = ctx.enter_context(tc.tile_pool(name="small", bufs=2))
    ptp = ctx.enter_context(tc.tile_pool(name="ptp", bufs=1))

    # ---- persistent SBUF tiles ----
    if FP8_QK:
        # [256-e-chunk][e_part, pair, k]; logical e = 256*c + 128*i + p
        xT8 = [const.tile([P, 2, S], FP8, name=f"xT8{c}", tag=f"xT8{c}")
               for c in range(NEC // 2)]
    else:
        xT = [const.tile([P, S], BF16, name=f"xT{e}", tag=f"xT{e}")
              for e in range(NEC)]                   # [E-chunk][e_part, k]
    V = [const.tile([P, E], BF16, name=f"V{kb}", tag=f"V{kb}")
         for kb in range(NKB)]                       # [k-block][k_part, e]
    sq_all = const.tile([P, NKB], F32, name="sq_all", tag="sq_all")
    biasK = const.tile([P, NKB], F32, name="biasK", tag="biasK")
    sqq_sc = const.tile([P, NKB_SELF], BF16, name="sqq_sc", tag="sqq_sc")
    bcastQ = const.tile([P, SQ], BF16, name="bcastQ", tag="bcastQ")
    ones = const.tile([P, 1], BF16, name="ones", tag="ones")
    nc.vector.memset(ones, 1.0)
    ident = const.tile([P, P], BF16, name="ident", tag="ident")
    make_identity(nc, ident)

    # ---- prologue: load x, compute ||x||^2, cast to bf16, build x^T ----
    # Transposes run on the (otherwise idle) PE via identity matmuls; DVE
    # drains them from PSUM into the xT tiles.
    with tc.tile_pool(name="sq_ps", bufs=2, space="PSUM") as sq_ps, \
         tc.tile_pool(name="tr_ps", bufs=4, space="PSUM") as tr_ps:
        for kb in range(NKB):
            src = xs_d if kb < NKB_SELF else xo_d
            r0 = (kb % NKB_SELF) * P
            xst = stage.tile([P, E], F32, name="xst", tag="xst")
            nc.sync.dma_start(out=xst, in_=src[r0:r0 + P, :])
            # sum of squares per row via ACT accumulate (squares discarded)
            sqt = sq_ps.tile([P, E], F32, name="sqt", tag="sqt")
            nc.scalar.activation(sqt, xst, AF.Square,
                                 accum_out=sq_all[:, kb:kb + 1])
            nc.scalar.copy(V[kb], xst)               # f32 -> bf16 cast
            for e in range(NEC):
                trp = tr_ps.tile([P, P], BF16, name="trp", tag="trp")
                nc.tensor.transpose(trp, V[kb][:, e * P:(e + 1) * P], ident)
                if FP8_QK:
                    c, i = divmod(e, 2)
                    nc.vector.tensor_copy(
                        xT8[c][:, i, kb * P:(kb + 1) * P], trp)
                else:
                    nc.vector.tensor_copy(xT[e][:, kb * P:(kb + 1) * P], trp)

    nc.vector.tensor_scalar_mul(biasK, sq_all, -gamma)
    nc.vector.tensor_scalar_mul(sqq_sc, sq_all[:, :NKB_SELF], -0.5)
    # roundtrip through DRAM to broadcast -0.5*||x_q||^2 along partitions
    nc.sync.dma_start(out=sqq_d[:].rearrange("(c p) -> p c", p=P), in_=sqq_sc)
    s_ap = sqq_d[:]
    bq_src = bass.AP(tensor=s_ap.tensor, offset=s_ap.offset,
                     ap=[[0, P]] + list(s_ap.ap))
    nc.sync.dma_start(out=bcastQ, in_=bq_src)

    # ---- main loop: PSUM pools (8 banks total: 2 + 4 + 2) ----
    qk_ps = ctx.enter_context(tc.tile_pool(name="qk_ps", bufs=2, space="PSUM"))
    out_ps = ctx.enter_context(tc.tile_pool(name="out_ps", bufs=2, space="PSUM"))
    s_ps = ctx.enter_context(tc.tile_pool(name="s_ps", bufs=2, space="PSUM"))

    for qb in range(NQB):
        q0 = qb * QB
        # Phase A: P^T[k, q0:q0+QB] for all 32 key blocks
        pts = []
        for kb in range(NKB):
            qkp = qk_ps.tile([P, QB], F32, name="qkp", tag="qkp")
            if FP8_QK:
                for c in range(NEC // 2):
                    nc.tensor.matmul(qkp,
                                     lhsT=xT8[c][:, :, kb * P:(kb + 1) * P],
                                     rhs=xT8[c][:, :, q0:q0 + QB],
                                     start=(c == 0), stop=(c == NEC // 2 - 1),
                                     perf_mode=mybir.MatmulPerfMode.DoubleRow)
            else:
                for e in range(NEC):
                    nc.tensor.matmul(qkp,
                                     lhsT=xT[e][:, kb * P:(kb + 1) * P],
                                     rhs=xT[e][:, q0:q0 + QB],
                                     start=(e == 0), stop=(e == NEC - 1))
            tt = tpool.tile([P, QB], F32, name="tt", tag="tt")
            nc.vector.tensor_add(tt, qkp, bcastQ[:, q0:q0 + QB])
            pt = ptp.tile([P, QB], BF16, name=f"pt{kb}", tag=f"pt{kb}")
            nc.scalar.activation(pt, tt, AF.Exp,
                                 bias=biasK[:, kb:kb + 1], scale=2.0 * gamma)
            pts.append(pt)
        # Phase B: out[q, :] = (P^T)^T @ V, row-sum via ones column
        for qs in range(NQS):
            po = out_ps.tile([P, E], F32, name="po", tag="po")
            sp = s_ps.tile([P, 1], F32, name="sp", tag="sp")
            for kb in range(NKB):
                lw = pts[kb][:, qs * P:(qs + 1) * P]
                nc.tensor.matmul(po[:, 0:EH], lhsT=lw, rhs=V[kb][:, 0:EH],
                                 start=(kb == 0), stop=(kb == NKB - 1))
                nc.tensor.matmul(po[:, EH:E], lhsT=lw, rhs=V[kb][:, EH:E],
                                 start=(kb == 0), stop=(kb == NKB - 1))
                nc.tensor.matmul(sp, lhsT=lw, rhs=ones,
                                 start=(kb == 0), stop=(kb == NKB - 1))
            rc = small.tile([P, 1], F32, name="rc", tag="rc")
            nc.vector.reciprocal(rc, sp)
            ot = opool.tile([P, E], F32, name="ot", tag="ot")
            nc.vector.tensor_scalar_mul(ot, po, rc)
            nc.sync.dma_start(out=out_d[q0 + qs * P:q0 + (qs + 1) * P, :],
                              in_=ot)


def build_module(gamma: float):
    nc = bacc.Bacc("TRN2", target_bir_lowering=False, debug=False)
    xs_d = nc.dram_tensor("x_self", [SQ, E], F32, kind="ExternalInput")
    xo_d = nc.dram_tensor("x_other", [SQ, E], F32, kind="ExternalInput")
    out_d = nc.dram_tensor("out", [SQ, E], F32, kind="ExternalOutput")
    sqq_d = nc.dram_tensor("sqq_scratch", [SQ], BF16)
    with tile.TileContext(nc) as tc, ExitStack() as ctx:
        _build_body(ctx, tc, gamma, xs_d, xo_d, out_d, sqq_d)
    nc.compile()
    return nc


_CACHE: dict[float, object] = {}


def _get_module(gamma: float):
    if gamma not in _CACHE:
        _CACHE[gamma] = build_module(gamma)
    return _CACHE[gamma]


def kernel(x, gamma):
    x = np.ascontiguousarray(np.asarray(x, dtype=np.float32))
    g = float(np.asarray(gamma))
    nc = _get_module(g)
    in_maps = []
    for c in range(NCORES):
        b, h = divmod(c, 2)
        xs = np.ascontiguousarray(x[b, h * SQ:(h + 1) * SQ])
        xo = np.ascontiguousarray(x[b, (1 - h) * SQ:(2 - h) * SQ])
        in_maps.append({"x_self": xs, "x_other": xo})
    res = run_bass_kernel_spmd(nc, in_maps, list(range(NCORES))).results
    out = np.empty((B, S, E), np.float32)
    for c in range(NCORES):
        b, h = divmod(c, 2)
        out[b, h * SQ:(h + 1) * SQ] = res[c]["out"]
    return out


if __name__ == "__main__":
    xs = np.random.randn(B, S, E).astype(np.float32)
    o = kernel(xs, np.float32(1.0))
    print("ran", o.shape, o.dtype)


# revision 8
# speedup vs baseline: 2.1259x; 1.0014x over previous
"""RBF kernel attention (nn_KernelAttention) on 8 Trainium2 NeuronCores.

reference math (per batch b):
    dist2[i,j] = ||x_i||^2 + ||x_j||^2 - 2 x_i.x_j
    attn = softmax(-gamma * max(dist2, 0), axis=j)
    out  = attn @ x

Key structural facts used here:
  * For RBF attention the diagonal logit is always ~0 and all logits are
    <= 0 (dist2 >= 0), so no separate row-max pass is needed for a safe
    exp() -- we exponentiate -gamma*dist2 directly and normalize by the
    row sum, exactly like the reference (which subtracts a row max of 0).
  * softmax is invariant to a per-row (per-query) additive constant, so
    the -gamma*||x_q||^2 term only needs enough precision to prevent
    overflow; bf16 is plenty (any rounding cancels between P and sum(P)).
  * We compute the score matrix transposed, L^T[k, q], so the exp output
    P^T is directly the stationary (lhsT) operand of the P @ V matmul --
    no on-chip transpose of the attention matrix is ever needed.

Sharding: core c handles batch c//2, query half c%2 (2048 queries),
against the batch's full 4096 keys. No collectives; host concatenates.

SPMD trick: every core receives x_self (its own query rows, also the
first half of its key/value set) and x_other (the remaining rows).
Attention is key-order invariant, so "self keys first" is fine and all
cores run the identical program.
"""

import sys

if "/opt/trn_rl_repo" not in sys.path:
    sys.path.insert(0, "/opt/trn_rl_repo")

from contextlib import ExitStack

import numpy as np

import concourse.bass as bass
import concourse.mybir as mybir
import concourse.tile as tile
from concourse import bacc
from concourse.bass_utils import run_bass_kernel_spmd
from concourse.masks import make_identity

F32 = mybir.dt.float32
BF16 = mybir.dt.bfloat16
FP8 = mybir.dt.float8e4
AF = mybir.ActivationFunctionType

FP8_QK = True   # fp8 DoubleRow for the Q@K^T gram matmul (2x PE rate)

B, S, E = 4, 4096, 1024
NCORES = 8
P = 128                 # partitions
SQ = S // 2             # queries per core
NKB = S // P            # 32 key blocks
NKB_SELF = SQ // P      # 16 key blocks coming from x_self
NEC = E // P            # 8 contraction chunks for Q@K^T
QB = 512                # query free-dim tile for QK / exp
NQB = SQ // QB          # 4
NQS = QB // P           # 4 query subtiles per query block
EH = 512                # PV free-dim half (PSUM bank limit)


def _build_body(ctx: ExitStack, tc: tile.TileContext, gamma: float,
                xs_d, xo_d, out_d, sqq_d):
    nc = tc.nc

    const = ctx.enter_context(tc.tile_pool(name="const", bufs=1))
    stage = ctx.enter_context(tc.tile_pool(name="stage", bufs=4))
    tpool = ctx.enter_context(tc.tile_pool(name="tpool", bufs=3))
    opool = ctx.enter_context(tc.tile_pool(name="opool", bufs=2))
    small = ctx.enter_context(tc.tile_pool(name="small", bufs=2))
    ptp = ctx.enter_context(tc.tile_pool(name="ptp", bufs=1))

    # ---- persistent SBUF tiles ----
    if FP8_QK:
        # [256-e-chunk][e_part, pair, k]; logical e = 256*c + 128*i + p
        xT8 = [const.tile([P, 2, S], FP8, name=f"xT8{c}", tag=f"xT8{c}")
               for c in range(NEC // 2)]
    else:
        xT = [const.tile([P, S], BF16, name=f"xT{e}", tag=f"xT{e}")
              for e in range(NEC)]                   # [E-chunk][e_part, k]
    V = [const.tile([P, E], BF16, name=f"V{kb}", tag=f"V{kb}")
         for kb in range(NKB)]                       # [k-block][k_part, e]
    sq_all = const.tile([P, NKB], F32, name="sq_all", tag="sq_all")
    biasK = const.tile([P, NKB], F32, name="biasK", tag="biasK")
    sqq_sc = const.tile([P, NKB_SELF], BF16, name="sqq_sc", tag="sqq_sc")
    bcastQ = const.tile([P, SQ], BF16, name="bcastQ", tag="bcastQ")
    ones = const.tile([P, 1], BF16, name="ones", tag="ones")
    nc.vector.memset(ones, 1.0)
    ident = const.tile([P, P], BF16, name="ident", tag="ident")
    make_identity(nc, ident)

    # ---- prologue: load x, compute ||x||^2, cast to bf16, build x^T ----
    # Transposes run on the (otherwise idle) PE via identity matmuls; DVE
    # drains them from PSUM into the xT tiles.
    with tc.tile_pool(name="sq_ps", bufs=2, space="PSUM") as sq_ps, \
         tc.tile_pool(name="tr_ps", bufs=4, space="PSUM") as tr_ps:
        for kb in range(NKB):
            src = xs_d if kb < NKB_SELF else xo_d
            r0 = (kb % NKB_SELF) * P
            xst = stage.tile([P, E], F32, name="xst", tag="xst")
            nc.sync.dma_start(out=xst, in_=src[r0:r0 + P, :])
            nc.scalar.copy(V[kb], xst)               # f32 -> bf16 cast
            # sum of squares per row via ACT accumulate (squares discarded)
            sqt = sq_ps.tile([P, E], F32, name="sqt", tag="sqt")
            nc.scalar.activation(sqt, xst, AF.Square,
                                 accum_out=sq_all[:, kb:kb + 1])
            for e in range(NEC):
                trp = tr_ps.tile([P, P], BF16, name="trp", tag="trp")
                nc.tensor.transpose(trp, V[kb][:, e * P:(e + 1) * P], ident)
                if FP8_QK:
                    c, i = divmod(e, 2)
                    nc.vector.tensor_copy(
                        xT8[c][:, i, kb * P:(kb + 1) * P], trp)
                else:
                    nc.vector.tensor_copy(xT[e][:, kb * P:(kb + 1) * P], trp)
            if kb == NKB_SELF - 1:
                # self-half stats ready: unblock exp biases + bcastQ early
                nc.vector.tensor_scalar_mul(
                    biasK[:, :NKB_SELF], sq_all[:, :NKB_SELF], -gamma)
                nc.vector.tensor_scalar_mul(
                    sqq_sc, sq_all[:, :NKB_SELF], -0.5)
                nc.sync.dma_start(
                    out=sqq_d[:].rearrange("(c p) -> p c", p=P), in_=sqq_sc)
                s_ap = sqq_d[:]
                bq_src = bass.AP(tensor=s_ap.tensor, offset=s_ap.offset,
                                 ap=[[0, P]] + list(s_ap.ap))
                nc.sync.dma_start(out=bcastQ, in_=bq_src)

    nc.vector.tensor_scalar_mul(biasK[:, NKB_SELF:], sq_all[:, NKB_SELF:],
                                -gamma)

    # ---- main loop: PSUM pools (8 banks total: 2 + 4 + 2) ----
    qk_ps = ctx.enter_context(tc.tile_pool(name="qk_ps", bufs=2, space="PSUM"))
    out_ps = ctx.enter_context(tc.tile_pool(name="out_ps", bufs=2, space="PSUM"))
    s_ps = ctx.enter_context(tc.tile_pool(name="s_ps", bufs=2, space="PSUM"))

    for qb in range(NQB):
        q0 = qb * QB
        # Phase A: P^T[k, q0:q0+QB] for all 32 key blocks
        pts = []
        for kb in range(NKB):
            qkp = qk_ps.tile([P, QB], F32, name="qkp", tag="qkp")
            if FP8_QK:
                for c in range(NEC // 2):
                    nc.tensor.matmul(qkp,
                                     lhsT=xT8[c][:, :, kb * P:(kb + 1) * P],
                                     rhs=xT8[c][:, :, q0:q0 + QB],
                                     start=(c == 0), stop=(c == NEC // 2 - 1),
                                     perf_mode=mybir.MatmulPerfMode.DoubleRow)
            else:
                for e in range(NEC):
                    nc.tensor.matmul(qkp,
                                     lhsT=xT[e][:, kb * P:(kb + 1) * P],
                                     rhs=xT[e][:, q0:q0 + QB],
                                     start=(e == 0), stop=(e == NEC - 1))
            tt = tpool.tile([P, QB], F32, name="tt", tag="tt")
            nc.vector.tensor_add(tt, qkp, bcastQ[:, q0:q0 + QB])
            pt = ptp.tile([P, QB], BF16, name=f"pt{kb}", tag=f"pt{kb}")
            nc.scalar.activation(pt, tt, AF.Exp,
                                 bias=biasK[:, kb:kb + 1], scale=2.0 * gamma)
            pts.append(pt)
        # Phase B: out[q, :] = (P^T)^T @ V, row-sum via ones column
        for qs in range(NQS):
            po = out_ps.tile([P, E], F32, name="po", tag="po")
            sp = s_ps.tile([P, 1], F32, name="sp", tag="sp")
            for kb in range(NKB):
                lw = pts[kb][:, qs * P:(qs + 1) * P]
                nc.tensor.matmul(po[:, 0:EH], lhsT=lw, rhs=V[kb][:, 0:EH],
                                 start=(kb == 0), stop=(kb == NKB - 1))
                nc.tensor.matmul(po[:, EH:E], lhsT=lw, rhs=V[kb][:, EH:E],
                                 start=(kb == 0), stop=(kb == NKB - 1))
                nc.tensor.matmul(sp, lhsT=lw, rhs=ones,
                                 start=(kb == 0), stop=(kb == NKB - 1))
            rc = small.tile([P, 1], F32, name="rc", tag="rc")
            nc.vector.reciprocal(rc, sp)
            ot = opool.tile([P, E], F32, name="ot", tag="ot")
            nc.vector.tensor_scalar_mul(ot, po, rc)
            nc.sync.dma_start(out=out_d[q0 + qs * P:q0 + (qs + 1) * P, :],
                              in_=ot)


def build_module(gamma: float):
    nc = bacc.Bacc("TRN2", target_bir_lowering=False, debug=False)
    xs_d = nc.dram_tensor("x_self", [SQ, E], F32, kind="ExternalInput")
    xo_d = nc.dram_tensor("x_other", [SQ, E], F32, kind="ExternalInput")
    out_d = nc.dram_tensor("out", [SQ, E], F32, kind="ExternalOutput")
    sqq_d = nc.dram_tensor("sqq_scratch", [SQ], BF16)
    with tile.TileContext(nc) as tc, ExitStack() as ctx:
        _build_body(ctx, tc, gamma, xs_d, xo_d, out_d, sqq_d)
    nc.compile()
    return nc


_CACHE: dict[float, object] = {}


def _get_module(gamma: float):
    if gamma not in _CACHE:
        _CACHE[gamma] = build_module(gamma)
    return _CACHE[gamma]


def kernel(x, gamma):
    x = np.ascontiguousarray(np.asarray(x, dtype=np.float32))
    g = float(np.asarray(gamma))
    nc = _get_module(g)
    in_maps = []
    for c in range(NCORES):
        b, h = divmod(c, 2)
        xs = np.ascontiguousarray(x[b, h * SQ:(h + 1) * SQ])
        xo = np.ascontiguousarray(x[b, (1 - h) * SQ:(2 - h) * SQ])
        in_maps.append({"x_self": xs, "x_other": xo})
    res = run_bass_kernel_spmd(nc, in_maps, list(range(NCORES))).results
    out = np.empty((B, S, E), np.float32)
    for c in range(NCORES):
        b, h = divmod(c, 2)
        out[b, h * SQ:(h + 1) * SQ] = res[c]["out"]
    return out


if __name__ == "__main__":
    xs = np.random.randn(B, S, E).astype(np.float32)
    o = kernel(xs, np.float32(1.0))
    print("ran", o.shape, o.dtype)
